# revision 1
# baseline (speedup 1.0000x reference)
"""Trainium2 Bass kernel for nn_MinLoss_12343736009330.

Math: the reference loss is
    loss = sum_{b,s} || pf[b,s] - gf[b,match[b,s]] ||_2
where pf/gf are the per-(batch, source) flattened [L=T*D] signals, and match
is a greedy assignment on the 4x4 Euclidean cdist.  Since
    ||pf[s] - gf[m]||^2 = pn[s] + gn[m] - 2 <pf[s], gf[m]>,
the whole computation reduces to the per-batch 8x8 Gram matrix of the
8 vectors {pf[0..4], gf[0..4]} plus a tiny 4x4 greedy matching.

Sharding: batch axis (16) across 8 cores -> 2 batches/core.  Each core
computes its 2 Gram matrices on the TensorEngine (contraction over t in
128-row tiles; operand columns interleaved (d, v) so that the 8x8 diagonal
blocks of each accumulated matmul hold per-d-slice Gram contributions),
extracts + reduces the diagonal blocks with selector matmuls, computes
the squared distances on VectorE, runs the greedy matching on-device
(min -> one-hot mask -> penalty-table row -> mask row+column), and writes
the 8 greedy minima.  Host applies sqrt and sums across cores.
"""

import os
import sys

import numpy as np

try:
    import concourse.bass as bass  # noqa: F401
except ImportError:
    sys.path.insert(0, "/opt/trn_rl_repo")

import concourse.bass as bass
import concourse.tile as tile
from concourse import bacc, mybir
from concourse.bass_utils import run_bass_kernel_spmd


def _install_ntff_hook_shim():
    """The bare agent image lacks ``antenv.axon_hooks``, so trace=True under
    axon would ImportError.  Recreate the module with the ctypes-based NTFF
    hook from trn_agent_boot (degrades to hook=None if unavailable)."""
    import types

    try:
        import antenv.axon_hooks  # noqa: F401

        return
    except ImportError:
        pass
    hook = None
    try:
        from trn_agent_boot.trn_boot import _ntff_profile_via_ctypes

        so_path = "/opt/axon/libaxon_pjrt.so"
        if os.path.exists(so_path):
            hook = _ntff_profile_via_ctypes(so_path)
    except Exception:
        hook = None
    import antenv

    mod = types.ModuleType("antenv.axon_hooks")
    mod.get_axon_ntff_profile_hook = lambda: hook  # type: ignore[attr-defined]

    def _set(h):
        nonlocal hook
        hook = h

    mod.set_axon_ntff_profile_hook = _set  # type: ignore[attr-defined]
    sys.modules["antenv.axon_hooks"] = mod
    antenv.axon_hooks = mod


_install_ntff_hook_shim()

F32 = mybir.dt.float32

S, T, B, D = 4, 512, 16, 512
N_CORES = 8
NB = B // N_CORES          # batches per core
NTB = T // 128             # t-blocks per batch
NV = 2 * S                 # 8 vectors per batch (4 preds + 4 gts)

# "bf16": interleave-copy casts to bf16; matmuls at 1 cyc/col (fp32 PSUM).
# "fp32": interleave-copy stays fp32; matmuls at 4 cyc/col (exact).
VARIANT = os.environ.get("MINLOSS_VARIANT", "bf16")
BIG = 1.0e30


def _build_consts() -> np.ndarray:
    """Host-side constant block, DMA'd once: [128, 400] fp32.

    row 0, cols 0:256: penalty table TBL[j*16+k] = BIG if entries j and k
    of the flattened 4x4 dist matrix share a row or column.
    rows 0..8, cols 256:264: 8x8 identity (flatten matmuls).
    cols 264:392: 128x128 identity (diagonal-block selector matmuls).
    """
    c = np.zeros((128, 400), np.float32)
    idx = np.arange(256)
    jj, kk = idx // 16, idx % 16
    c[0, 0:256] = np.where((jj // 4 == kk // 4) | (jj % 4 == kk % 4), BIG, 0.0)
    c[0:8, 256:264] = np.eye(8, dtype=np.float32)
    c[:, 264:392] = np.eye(128, dtype=np.float32)
    return c


CONSTS = _build_consts()


def build_nc(variant: str = VARIANT):
    nc = bacc.Bacc(
        "TRN2",
        target_bir_lowering=False,
        debug=False,
        enable_asserts=True,
        num_devices=N_CORES,
    )
    # xa: host-side pre-interleaved shard.  xa[b, tb, p, g*128 + i*8 + v]
    # holds vector v's value at t = 128*tb + p, d = 16*g + i (v 0..3 preds,
    # 4..7 gts).  Each matmul operand is then one contiguous [128,128] slab.
    xa_t = nc.dram_tensor(
        "xa", [NB, NTB, 128, NV * D], F32, kind="ExternalInput"
    ).ap()
    consts_t = nc.dram_tensor("consts", [128, 400], F32, kind="ExternalInput").ap()
    # the 8 greedy minima (squared distances); host does sqrt + sum
    loss_t = nc.dram_tensor("loss", [1, 2 * S], F32, kind="ExternalOutput").ap()
    gram_t = nc.dram_tensor("gram", [NB, 64], F32, kind="ExternalOutput").ap()

    with tile.TileContext(nc) as tc:
        _build_tile(tc, xa_t, consts_t, loss_t, gram_t, variant)

    nc.compile()
    return nc


def _build_tile(tc, xa_t, consts_t, loss_t, gram_t, variant):
    nc = tc.nc
    import contextlib

    ctx = contextlib.ExitStack()
    with ctx:
        a_pool = ctx.enter_context(tc.tile_pool(name="a", bufs=4))
        b_pool = ctx.enter_context(tc.tile_pool(name="b", bufs=2 * NTB))
        psum_pool = ctx.enter_context(tc.tile_pool(name="psum", bufs=2, space="PSUM"))
        psumf_pool = ctx.enter_context(tc.tile_pool(name="psumf", bufs=2, space="PSUM"))
        consts_pool = ctx.enter_context(tc.tile_pool(name="consts", bufs=1))
        small_pool = ctx.enter_context(tc.tile_pool(name="small", bufs=2))

        csb = consts_pool.tile([128, 400], F32)
        nc.sync.dma_start(out=csb[:, :], in_=consts_t[:, :])
        tbl16 = csb[0:1, 0:256].rearrange("p (j k) -> p j k", k=16)
        ident8 = csb[0:8, 256:264]
        ident128 = csb[:, 264:392]

        # the 8 greedy minima (squared dists); sqrt+sum at the very end
        loss4 = small_pool.tile([1, 2 * S], F32, tag="loss4")

        bdt = mybir.dt.bfloat16 if variant == "bf16" else F32

        # ======== phase 1: all loads + Gram matmuls (both batches) ========
        # Emitting every load/cast before any reduction keeps the in-order
        # DVE stream free of batch-0 reduction work while batch-1 tiles are
        # still arriving.
        dma_mode = os.environ.get("MINLOSS_DMA", "dual")
        hwdge_tiles = {
            "dual": {(0, 0), (0, 2), (1, 0), (1, 1)},
            "swdge": set(),
            "hw2": set(),
        }[dma_mode]
        warm_mm = int(os.environ.get("MINLOSS_WARM_MM", "0"))
        if warm_mm:
            warm_pool = ctx.enter_context(
                tc.tile_pool(name="warm", bufs=1, space="PSUM")
            )
            psum_warm = warm_pool.tile([128, 128], F32)
        psums = []
        for ib in range(NB):
            psum = psum_pool.tile([128, 128], F32)
            psums.append(psum)
            for tb in range(NTB):
                b_tb = b_pool.tile([128, NV * D], bdt)
                if variant == "bf16" and dma_mode == "hw2":
                    # Both HWDGE rings (sync + scalar issue separate HW
                    # queues); no gpsimd in the program at all.  The last
                    # two tiles are striped so cast+matmuls overlap the
                    # DMA tail.
                    eng = nc.sync if tb % 2 == 0 else nc.scalar
                    a_tb = a_pool.tile([128, NV * D], F32)
                    nstrip = 4 if (ib == NB - 1 and tb >= NTB - 2) else 1
                    q = NV * D // nstrip
                    for st in range(nstrip):
                        sl = slice(st * q, (st + 1) * q)
                        eng.dma_start(out=a_tb[:, sl], in_=xa_t[ib, tb, :, sl])
                        nc.vector.tensor_copy(out=b_tb[:, sl], in_=a_tb[:, sl])
                elif variant == "bf16":
                    # Two concurrent DMA paths to saturate HBM (~358 GB/s):
                    # SWDGE casting DMA and HWDGE into an fp32 staging tile
                    # + DVE cast copy.  Late (critical-path) tiles always
                    # take the copy-free SWDGE path.
                    if (ib, tb) in hwdge_tiles:
                        a_tb = a_pool.tile([128, NV * D], F32)
                        nc.sync.dma_start(out=a_tb[:, :], in_=xa_t[ib, tb, :, :])
                        nc.vector.tensor_copy(out=b_tb[:, :], in_=a_tb[:, :])
                    elif ib == NB - 1 and tb >= NTB - 2:
                        # stripe the last tiles so their matmuls overlap the
                        # DMA tail
                        q = NV * D // 4
                        for st in range(4):
                            nc.gpsimd.dma_start(
                                out=b_tb[:, st * q : (st + 1) * q],
                                in_=xa_t[ib, tb, :, st * q : (st + 1) * q],
                            )
                    else:
                        nc.gpsimd.dma_start(out=b_tb[:, :], in_=xa_t[ib, tb, :, :])
                else:
                    nc.sync.dma_start(out=b_tb[:, :], in_=xa_t[ib, tb, :, :])

                for g in range(D // 16):
                    op = b_tb[:, g * 128 : (g + 1) * 128]
                    first = tb == 0 and g == 0
                    last = tb == NTB - 1 and g == D // 16 - 1
                    nc.tensor.matmul(
                        psum[:, :], lhsT=op, rhs=op, start=first, stop=last
                    )
                if warm_mm and not (ib == NB - 1 and tb == NTB - 1):
                    # keep the PE HAM-warm through the DMA wait for the next
                    # tile: junk matmuls reading this tile (the data dep pins
                    # them after the real burst in the PE stream)
                    wop = b_tb[:, 0:128]
                    for _ in range(warm_mm):
                        nc.tensor.matmul(
                            psum_warm[:, :],
                            lhsT=wop,
                            rhs=wop,
                            start=True,
                            stop=True,
                            skip_group_check=True,
                        )

        # ======== phase 2: per-batch reduction + matching ========
        for ib in range(NB):
            psum = psums[ib]
            # ---------------- diagonal-block reduction (on PE) ------------
            # Engine APs must start at 32-aligned partitions, so VectorE
            # cannot read the 8x8 blocks at partition 8q directly.  Instead
            # use selector matmuls: I128[:,8q:8q+8].T @ C[:,8q:8q+8] lands
            # block q on partitions 0:8, and PSUM accumulation sums over q.
            c_sb = small_pool.tile([128, 128], F32)
            nc.vector.tensor_copy(out=c_sb[:, :], in_=psum[:, :])
            psg = psumf_pool.tile([8, 8], F32, tag="psg")
            for q in range(16):
                nc.tensor.matmul(
                    psg[:, :],
                    lhsT=ident128[:, 8 * q : 8 * q + 8],
                    rhs=c_sb[:, 8 * q : 8 * q + 8],
                    start=(q == 0),
                    stop=(q == 15),
                )
            acc = small_pool.tile([8, 8], F32)
            nc.vector.tensor_copy(out=acc[:, :], in_=psg[:, :])

            # ---------------- flatten Gram to one partition ----------------
            psf = psumf_pool.tile([1, 72], F32)
            for p in range(8):
                nc.tensor.matmul(
                    psf[0:1, 8 * p : 8 * p + 8],
                    lhsT=ident8[:, p : p + 1],
                    rhs=acc[:, :],
                    start=True,
                    stop=True,
                )

            flat = small_pool.tile([1, 72], F32)
            nc.vector.tensor_copy(out=flat[0:1, 0:64], in_=psf[0:1, 0:64])
            nc.sync.dma_start(out=gram_t[ib : ib + 1, :], in_=flat[0:1, 0:64])

            # ------------- d2 = pn + gn - 2*cross (squared dists) --------
            # (sqrt is monotone, so the greedy matching runs on d2; the
            # sqrt of the 8 collected minima happens once at the end)
            g9 = flat[0:1, 0:72].rearrange("p (a b) -> p a b", b=9)
            pn = g9[:, 0:4, 0:1].broadcast_to((1, 4, 4))
            gn = g9[:, 4:8, 0:1].transpose([0, 2, 1]).broadcast_to((1, 4, 4))
            cross = flat[0:1, 0:64].rearrange("p (a b) -> p a b", b=8)[:, 0:4, 4:8]

            d2 = small_pool.tile([1, 16], F32)
            d2v = d2[0:1, :].rearrange("p (a b) -> p a b", b=4)
            tmp16 = small_pool.tile([1, 16], F32)
            tmp16v = tmp16[0:1, :].rearrange("p (a b) -> p a b", b=4)

            nc.vector.tensor_add(out=d2v, in0=pn, in1=gn)
            nc.vector.tensor_scalar(
                out=tmp16v,
                in0=cross,
                scalar1=-2.0,
                scalar2=None,
                op0=mybir.AluOpType.mult,
            )
            nc.vector.tensor_add(out=d2[:, :], in0=d2[:, :], in1=tmp16[:, :])

            # ---------------- greedy matching on d2 ----------------
            # per iteration: min -> one-hot mask of the argmin -> penalty
            # row from the table (max over the masked table) -> mask out
            # its row+column.  (On an exact fp32 tie both tied entries are
            # masked; the resulting loss difference is O(tie gap).)
            mask16 = small_pool.tile([1, 16], F32)
            cmp256 = small_pool.tile([1, 256], F32)
            pen = small_pool.tile([1, 16], F32)

            for it in range(S):
                slot = loss4[0:1, ib * S + it : ib * S + it + 1]
                nc.vector.tensor_reduce(
                    out=slot,
                    in_=d2[:, :],
                    axis=mybir.AxisListType.X,
                    op=mybir.AluOpType.min,
                )
                if it == S - 1:
                    break
                nc.vector.tensor_scalar(
                    out=mask16[:, :],
                    in0=d2[:, :],
                    scalar1=slot,
                    scalar2=None,
                    op0=mybir.AluOpType.is_le,
                )
                nc.vector.tensor_mul(
                    out=cmp256[0:1, :].rearrange("p (j k) -> p j k", k=16),
                    in0=tbl16,
                    in1=mask16[0:1, :].unsqueeze(1).broadcast_to((1, 16, 16)),
                )
                nc.vector.tensor_reduce(
                    out=pen[:, :],
                    in_=cmp256[0:1, :].rearrange("p (j k) -> p j k", k=16),
                    axis=mybir.AxisListType.X,
                    op=mybir.AluOpType.max,
                )
                nc.vector.tensor_add(out=d2[:, :], in0=d2[:, :], in1=pen[:, :])

        nc.sync.dma_start(out=loss_t[0:1, :], in_=loss4[:, :])


_NC_CACHE: dict = {}


def _get_nc(variant: str = VARIANT):
    key = (
        variant,
        os.environ.get("MINLOSS_DMA", "dual"),
        os.environ.get("MINLOSS_WARM_MM", "0"),
    )
    if key not in _NC_CACHE:
        _NC_CACHE[key] = build_nc(variant)
    return _NC_CACHE[key]


def shard_inputs(preds: np.ndarray, gts: np.ndarray):
    """Build the interleaved layout X[b, tb, p, g*128 + i*8 + v] and slice
    per core (b is outermost, so per-core slices are contiguous views)."""
    X = np.empty((B, NTB, 128, 32, 16, NV), np.float32)
    # preds [S, T, B, D] -> [b, tb, p, g, i, s]
    X[..., 0:S] = preds.reshape(S, NTB, 128, B, 32, 16).transpose(3, 1, 2, 4, 5, 0)
    # gts [S, B, T, D] -> [b, tb, p, g, i, s]
    X[..., S : 2 * S] = gts.reshape(S, B, NTB, 128, 32, 16).transpose(
        1, 2, 3, 4, 5, 0
    )
    X = X.reshape(B, NTB, 128, NV * D)
    in_maps = []
    for c in range(N_CORES):
        b0 = c * NB
        in_maps.append({"xa": X[b0 : b0 + NB], "consts": CONSTS})
    return in_maps


kernel_last_results = None


def kernel(preds: np.ndarray, gts: np.ndarray) -> np.ndarray:
    global kernel_last_results
    nc = _get_nc()
    in_maps = shard_inputs(np.asarray(preds), np.asarray(gts))
    trace = os.environ.get("MINLOSS_TRACE", "1") == "1"
    try:
        res = run_bass_kernel_spmd(
            nc, in_maps, core_ids=list(range(N_CORES)), trace=trace
        )
    except Exception:
        if not trace:
            raise
        # profiling infrastructure may be unavailable; rerun without it
        res = run_bass_kernel_spmd(
            nc, in_maps, core_ids=list(range(N_CORES)), trace=False
        )
    kernel_last_results = res
    total = 0.0
    for c in range(N_CORES):
        m2 = np.asarray(res.results[c]["loss"], dtype=np.float64)
        total += float(np.sqrt(np.maximum(m2, 0.0)).sum())
    return np.array(total, dtype=np.float32)



# revision 2
# speedup vs baseline: 1.9055x; 1.9055x over previous
"""Trainium2 Bass kernel for nn_MinLoss_12343736009330 (v2: fp8 DoubleRow).

Math: the reference loss is
    loss = sum_{b,s} || pf[b,s] - gf[b,match[b,s]] ||_2
where pf/gf are the per-(batch, source) flattened [L=T*D] signals, and match
is a greedy assignment on the 4x4 Euclidean cdist.  Since
    ||pf[s] - gf[m]||^2 = pn[s] + gn[m] - 2 <pf[s], gf[m]>,
the whole computation reduces to the per-batch 8x8 Gram matrix of the
8 vectors {pf[0..4], gf[0..4]} plus a tiny 4x4 greedy matching.

Key change vs v1: the host pre-casts the interleaved operand layout to
fp8 e4m3 (loss rel-err from input rounding ~4e-4, tolerance is 2e-2), which
cuts HBM traffic 4x vs fp32, and the Gram matmuls run in DoubleRow perf
mode (2 contraction rows per PE column-cycle).  DMA becomes the roofline:
4 MiB/core at ~332 GB/s ~= 12.6 us.

Sharding: batch axis (16) across 8 cores -> 2 batches/core.  Per batch the
t=512 contraction is covered by 2 tiles of [128 partitions, 2 (DoubleRow
halves), 4096 interleaved columns]; columns interleave (d-slice, vector) so
the 16 8x8 diagonal blocks of each accumulated 128x128 matmul hold per-
d-slice Gram contributions.  Selector matmuls reduce the diagonal blocks,
flatten matmuls put the Gram on one partition, and the greedy matching runs
on-device (min -> one-hot mask -> row/col conflict flags -> additive BIG
penalty).  The 8 greedy minima (squared) go back; host does sqrt + sum.
"""

import os
import sys

import ml_dtypes
import numpy as np

try:
    import concourse.bass as bass  # noqa: F401
except ImportError:
    sys.path.insert(0, "/opt/trn_rl_repo")

import concourse.bass as bass  # noqa: F811
import concourse.tile as tile
from concourse import bacc, mybir
from concourse.bass_utils import run_bass_kernel_spmd


def _install_ntff_hook_shim():
    """The bare agent image lacks ``antenv.axon_hooks``, so trace=True under
    axon would ImportError.  Recreate the module with the ctypes-based NTFF
    hook from trn_agent_boot (degrades to hook=None if unavailable)."""
    import types

    try:
        import antenv.axon_hooks  # noqa: F401

        return
    except ImportError:
        pass
    hook = None
    try:
        from trn_agent_boot.trn_boot import _ntff_profile_via_ctypes

        so_path = "/opt/axon/libaxon_pjrt.so"
        if os.path.exists(so_path):
            hook = _ntff_profile_via_ctypes(so_path)
    except Exception:
        hook = None
    import antenv

    mod = types.ModuleType("antenv.axon_hooks")
    mod.get_axon_ntff_profile_hook = lambda: hook  # type: ignore[attr-defined]

    def _set(h):
        nonlocal hook
        hook = h

    mod.set_axon_ntff_profile_hook = _set  # type: ignore[attr-defined]
    sys.modules["antenv.axon_hooks"] = mod
    antenv.axon_hooks = mod


_install_ntff_hook_shim()

F32 = mybir.dt.float32
BF16 = mybir.dt.bfloat16

S, T, B, D = 4, 512, 16, 512
N_CORES = 8
NB = B // N_CORES          # batches per core
NTBP = 2                   # t-block pairs per batch (4 blocks of 128 rows)
NV = 2 * S                 # 8 vectors per batch (4 preds + 4 gts)
NCOL = NV * D              # 4096 interleaved columns per DoubleRow half
NG = NCOL // 128           # 32 column groups per half
BIG = 1.0e30

# "fp8": e4m3 operands + DoubleRow matmuls (2 rows/cycle).
# "bf16": bf16 operands, plain matmuls.  Host pre-casts either way.
VARIANT = os.environ.get("MINLOSS_VARIANT", "fp8")
NSTRIP = int(os.environ.get("MINLOSS_NSTRIP", "4"))
# DMA issue queues, round-robin per strip: s=sync a=scalar v=vector g=gpsimd
QUEUES = os.environ.get("MINLOSS_QUEUES", "sa")

ID8 = np.eye(8, dtype=np.float32)
ID128 = np.eye(128, dtype=ml_dtypes.bfloat16)


def _bass_dt(variant: str):
    return mybir.dt.float8e4 if variant == "fp8" else BF16


def _np_dt(variant: str):
    return ml_dtypes.float8_e4m3 if variant == "fp8" else ml_dtypes.bfloat16


def build_nc(variant: str, nstrip: int, queues: str):
    nc = bacc.Bacc(
        "TRN2",
        target_bir_lowering=False,
        debug=False,
        enable_asserts=True,
        num_devices=N_CORES,
    )
    bdt = _bass_dt(variant)
    # xa[b, tbp, p, i, g*128 + ii*8 + v]: vector v's value at
    # t = 256*tbp + 128*i + p, d = 16*g + ii (v 0..3 preds, 4..7 gts).
    xa_t = nc.dram_tensor(
        "xa", [NB, NTBP, 128, 2, NCOL], bdt, kind="ExternalInput"
    ).ap()
    id8_t = nc.dram_tensor("id8", [8, 8], F32, kind="ExternalInput").ap()
    id128_t = nc.dram_tensor("id128", [128, 128], BF16, kind="ExternalInput").ap()
    # the 8 greedy minima (squared distances); host does sqrt + sum
    loss_t = nc.dram_tensor("loss", [1, 2 * S], F32, kind="ExternalOutput").ap()

    with tile.TileContext(nc) as tc:
        _build_tile(tc, xa_t, id8_t, id128_t, loss_t, variant, nstrip, queues)

    nc.compile()
    return nc


def _build_tile(tc, xa_t, id8_t, id128_t, loss_t, variant, nstrip, queues):
    nc = tc.nc
    import contextlib

    ctx = contextlib.ExitStack()
    with ctx:
        b_pool = ctx.enter_context(tc.tile_pool(name="b", bufs=NB * NTBP))
        psum_pool = ctx.enter_context(tc.tile_pool(name="psum", bufs=NB, space="PSUM"))
        psumf_pool = ctx.enter_context(tc.tile_pool(name="psumf", bufs=2, space="PSUM"))
        consts_pool = ctx.enter_context(tc.tile_pool(name="consts", bufs=1))
        small_pool = ctx.enter_context(tc.tile_pool(name="small", bufs=2))

        bdt = _bass_dt(variant)
        qmap = {"s": nc.sync, "a": nc.scalar, "v": nc.vector, "g": nc.gpsimd}
        qlist = [qmap[ch] for ch in queues]

        id8 = consts_pool.tile([8, 8], F32, tag="id8")
        idb = consts_pool.tile([128, 128], BF16, tag="idb")
        nc.gpsimd.dma_start(out=id8[:, :], in_=id8_t[:, :])
        nc.gpsimd.dma_start(out=idb[:, :], in_=id128_t[:, :])

        # the 8 greedy minima (squared dists); sqrt+sum on host at the end
        loss4 = small_pool.tile([1, 2 * S], F32, tag="loss4")

        # ======== phase 0: issue ALL input strips up front ========
        # Strips round-robin over the issue queues so the DMA engines see a
        # continuous descriptor supply; tiles land roughly in order, matmuls
        # chase the strips.
        tiles = []
        qi = 0
        cw = NCOL // nstrip
        for ib in range(NB):
            for tbp in range(NTBP):
                btl = b_pool.tile([128, 2, NCOL], bdt, name=f"btl_{ib}_{tbp}")
                tiles.append(btl)
                for st in range(nstrip):
                    sl = slice(st * cw, (st + 1) * cw)
                    q = qlist[qi % len(qlist)]
                    qi += 1
                    q.dma_start(out=btl[:, :, sl], in_=xa_t[ib, tbp, :, :, sl])

        # ======== per batch: Gram matmuls + reduction + matching ========
        for ib in range(NB):
            psum = psum_pool.tile([128, 128], F32, name=f"psum_{ib}")
            for tbp in range(NTBP):
                btl = tiles[ib * NTBP + tbp]
                for g in range(NG):
                    first = tbp == 0 and g == 0
                    last = tbp == NTBP - 1 and g == NG - 1
                    if variant == "fp8":
                        op = btl[:, :, g * 128 : (g + 1) * 128]
                        nc.tensor.matmul(
                            psum[:, :],
                            lhsT=op,
                            rhs=op,
                            start=first,
                            stop=last,
                            perf_mode=mybir.MatmulPerfMode.DoubleRow,
                        )
                    else:
                        for i in range(2):
                            opi = btl[:, i, g * 128 : (g + 1) * 128]
                            nc.tensor.matmul(
                                psum[:, :],
                                lhsT=opi,
                                rhs=opi,
                                start=first and i == 0,
                                stop=last and i == 1,
                            )

            # ---------------- diagonal-block reduction (on PE) ------------
            # Engine APs can't start at partition 8q, so selector matmuls
            # I128[:,8q:8q+8].T @ C[:,8q:8q+8] move block q to partitions
            # 0:8 and PSUM-accumulate over q.  bf16 copy halves DVE/ident
            # cost; partial-Gram bf16 rounding is ~1e-4 on the loss.
            c_sb = small_pool.tile([128, 128], BF16, name=f"c_sb_{ib}")
            nc.vector.tensor_copy(out=c_sb[:, :], in_=psum[:, :])
            psg = psumf_pool.tile([8, 8], F32, tag="psg", name=f"psg_{ib}")
            for q in range(16):
                nc.tensor.matmul(
                    psg[:, :],
                    lhsT=idb[:, 8 * q : 8 * q + 8],
                    rhs=c_sb[:, 8 * q : 8 * q + 8],
                    start=(q == 0),
                    stop=(q == 15),
                )
            acc = small_pool.tile([8, 8], F32, name=f"acc_{ib}")
            nc.vector.tensor_copy(out=acc[:, :], in_=psg[:, :])

            # ---------------- flatten Gram to one partition ----------------
            psf = psumf_pool.tile([1, 72], F32, tag="psf", name=f"psf_{ib}")
            for p in range(8):
                nc.tensor.matmul(
                    psf[0:1, 8 * p : 8 * p + 8],
                    lhsT=id8[:, p : p + 1],
                    rhs=acc[:, :],
                    start=True,
                    stop=True,
                )

            # ------------- d2 = pn + gn - 2*cross, read from PSUM --------
            # (sqrt is monotone, so the greedy matching runs on d2; the
            # sqrt of the 8 collected minima happens on the host)
            g9 = psf[0:1, 0:72].rearrange("p (a b) -> p a b", b=9)
            pn = g9[:, 0:4, 0:1].broadcast_to((1, 4, 4))
            gn = g9[:, 4:8, 0:1].transpose([0, 2, 1]).broadcast_to((1, 4, 4))
            cross = psf[0:1, 0:64].rearrange("p (a b) -> p a b", b=8)[:, 0:4, 4:8]

            d2 = small_pool.tile([1, 16], F32, name=f"d2_{ib}")
            d2v = d2[0:1, :].rearrange("p (a b) -> p a b", b=4)
            nc.vector.tensor_scalar(
                out=d2v,
                in0=cross,
                scalar1=-2.0,
                scalar2=None,
                op0=mybir.AluOpType.mult,
            )
            nc.vector.tensor_add(out=d2v, in0=d2v, in1=pn)
            nc.vector.tensor_add(out=d2v, in0=d2v, in1=gn)

            # ---------------- greedy matching on d2 ----------------
            # per iteration: min -> {0,BIG} mask of the argmin -> row/col
            # conflict flags (max-reduce over the 4x4 mask) -> add both
            # flags into d2.  (On an exact fp32 tie both tied entries are
            # masked; the resulting loss difference is O(tie gap).)
            mask16 = small_pool.tile([1, 16], F32, name=f"mask16_{ib}")
            rc = small_pool.tile([1, 8], F32, name=f"rc_{ib}")
            m44 = mask16[0:1, :].rearrange("p (r c) -> p r c", c=4)
            m44t = m44.transpose([0, 2, 1])
            rcv = rc[0:1, :].rearrange("p (x y) -> p x y", y=4)
            rowb = rcv[:, 0:1, :].transpose([0, 2, 1]).broadcast_to((1, 4, 4))
            colb = rcv[:, 1:2, :].broadcast_to((1, 4, 4))

            for it in range(S):
                slot = loss4[0:1, ib * S + it : ib * S + it + 1]
                nc.vector.tensor_reduce(
                    out=slot,
                    in_=d2[:, :],
                    axis=mybir.AxisListType.X,
                    op=mybir.AluOpType.min,
                )
                if it == S - 1:
                    break
                nc.vector.tensor_scalar(
                    out=mask16[:, :],
                    in0=d2[:, :],
                    scalar1=slot,
                    scalar2=BIG,
                    op0=mybir.AluOpType.is_le,
                    op1=mybir.AluOpType.mult,
                )
                nc.vector.tensor_reduce(
                    out=rc[0:1, 0:4],
                    in_=m44,
                    axis=mybir.AxisListType.X,
                    op=mybir.AluOpType.max,
                )
                nc.vector.tensor_reduce(
                    out=rc[0:1, 4:8],
                    in_=m44t,
                    axis=mybir.AxisListType.X,
                    op=mybir.AluOpType.max,
                )
                nc.vector.tensor_add(out=d2v, in0=d2v, in1=rowb)
                nc.vector.tensor_add(out=d2v, in0=d2v, in1=colb)

        nc.sync.dma_start(out=loss_t[0:1, :], in_=loss4[:, :])


_NC_CACHE: dict = {}


def _get_nc():
    key = (VARIANT, NSTRIP, QUEUES)
    if key not in _NC_CACHE:
        _NC_CACHE[key] = build_nc(*key)
    return _NC_CACHE[key]


def shard_inputs(preds: np.ndarray, gts: np.ndarray, variant: str):
    """Build the interleaved low-precision layout
    X[b, tbp, p, i, g*128 + ii*8 + v] and slice per core (b outermost, so
    per-core slices are contiguous views)."""
    npdt = _np_dt(variant)
    p8 = np.asarray(preds).astype(npdt)
    g8 = np.asarray(gts).astype(npdt)
    X = np.empty((B, NTBP, 128, 2, 32, 16, NV), npdt)
    # preds [S, T, B, D] -> [b, tbp, p, i, g, ii, s]
    X[..., 0:S] = p8.reshape(S, 2, 2, 128, B, 32, 16).transpose(4, 1, 3, 2, 5, 6, 0)
    # gts [S, B, T, D] -> [b, tbp, p, i, g, ii, s]
    X[..., S : 2 * S] = g8.reshape(S, B, 2, 2, 128, 32, 16).transpose(
        1, 2, 4, 3, 5, 6, 0
    )
    X = X.reshape(B, NTBP, 128, 2, NCOL)
    in_maps = []
    for c in range(N_CORES):
        b0 = c * NB
        in_maps.append({"xa": X[b0 : b0 + NB], "id8": ID8, "id128": ID128})
    return in_maps


kernel_last_results = None


def kernel(preds: np.ndarray, gts: np.ndarray) -> np.ndarray:
    global kernel_last_results
    nc = _get_nc()
    in_maps = shard_inputs(preds, gts, VARIANT)
    trace = os.environ.get("MINLOSS_TRACE", "1") == "1"
    try:
        res = run_bass_kernel_spmd(
            nc, in_maps, core_ids=list(range(N_CORES)), trace=trace
        )
    except Exception:
        if not trace:
            raise
        # profiling infrastructure may be unavailable; rerun without it
        res = run_bass_kernel_spmd(
            nc, in_maps, core_ids=list(range(N_CORES)), trace=False
        )
    kernel_last_results = res
    total = 0.0
    for c in range(N_CORES):
        m2 = np.asarray(res.results[c]["loss"], dtype=np.float64)
        total += float(np.sqrt(np.maximum(m2, 0.0)).sum())
    return np.array(total, dtype=np.float32)
